# revision 18
# baseline (speedup 1.0000x reference)
"""Trainium2 Bass kernel for nn_CircumpunctAttention.

Full inputs in, full output out. Internally: data-parallel over batch (2) x
tensor-parallel over heads (4 head-groups of 4 heads) = 8 NeuronCores.

Per core the computation is plain multi-head attention on 4 heads:
  qT = (Wq/scale)_shard @ x_b^T          [256, 2048]   (dh on partitions)
  kT = Wk_shard @ x_b^T                  [256, 2048]
  v  = x_b @ Wv_shard^T (natural)        [2048, 256]   + ones column per head
  per head:  ST = K Q^T -> exp -> P;  outT = [V;1]^T P  (row 64 = softmax denom)
  normalize by reciprocal of denom row, then emerge matmul with the
  chamber-folded We shard produces the partial output [2048, 1024].

The per-head "aperture chamber" (input/output valves, rotation by pi*sigmoid
(beta), tanh(chi) gate) is a constant linear map on each head's 64 channels,
so it is folded into We host-side in float64. The softmax max-subtraction is
skipped: scores are bounded (|s| < ~7 for this problem's scale), so exp is
well within fp32 range and results match jax.nn.softmax to fp32 roundoff.
"""

import math
from contextlib import ExitStack
import numpy as np

# ---------------------------------------------------------------- constants
P = 128          # partitions
T = 2048         # sequence length
D = 1024         # model dim
H = 16           # total heads
DH = 64          # head dim
HC = 4           # heads per core
C = HC * DH      # channels per core (256)
KT = D // P      # 8 contraction tiles over model dim
TT = T // P      # 16 tiles over sequence
MT = C // P      # 2 partition tiles over per-core channels
NCORES = 8
SCALE = 8.0      # sqrt(dh * conv_factor), conv_factor = 1/phi^0 = 1

# dtype configuration for each matmul stage
CFG = {
    "dt_x": "bfloat16",    # xT / Wq / Wk / Wv storage + proj matmul dtype
    "dt_qk": "bfloat16",   # qT/kT storage -> scores matmul dtype
    "dt_p": "bfloat16",    # P = exp(S) and v_aug storage -> attnV matmul dtype
    "dt_o": "bfloat16",    # oT / We storage -> emerge matmul dtype
    "nch_bf16": 512,       # moving-operand chunk for bf16 matmuls
}

LAST_EXEC_NS = None
_CACHE = {}


def _np_dt(name):
    if name == "bfloat16":
        import ml_dtypes
        return np.dtype(ml_dtypes.bfloat16)
    return np.dtype(name)


def build_nc(cfg=CFG):
    """Build + compile the single-core SPMD program."""
    import concourse.bass as bass
    import concourse.mybir as mybir
    import concourse.tile as tile
    from concourse import bacc

    dt = mybir.dt
    f32 = dt.float32
    dtx = getattr(dt, cfg["dt_x"])
    dtqk = getattr(dt, cfg["dt_qk"])
    dtp = getattr(dt, cfg["dt_p"])
    dto = getattr(dt, cfg["dt_o"])

    def nch(d):
        return 512 if d == dt.float32 else cfg["nch_bf16"]

    nc = bacc.Bacc("TRN2", target_bir_lowering=False, debug=False,
                   enable_asserts=False)

    xT = nc.dram_tensor("xt", [D, T], dtx, kind="ExternalInput").ap()
    wq = nc.dram_tensor("wq", [D, C], dtx, kind="ExternalInput").ap()
    wk = nc.dram_tensor("wk", [D, C], dtx, kind="ExternalInput").ap()
    wv = nc.dram_tensor("wv", [D, C], dtx, kind="ExternalInput").ap()
    we = nc.dram_tensor("we", [C, D], dto, kind="ExternalInput").ap()
    out = nc.dram_tensor("out", [T, D], f32, kind="ExternalOutput").ap()

    Exp = mybir.ActivationFunctionType.Exp
    JW = 1024  # query-half width in the attention loop

    with tile.TileContext(nc) as tc, ExitStack() as ctx:
        # One PSUM pool layout for the whole kernel so projection, attention
        # and emerge phases can interleave: tag "s" ([128,1024] f32, 2 bufs,
        # 4 banks) is shared by q/k projection, scores and emerge matmuls;
        # tag "v" (1 bank x2) by the v projection; tag "o" ([128,1024], 1
        # buf, 2 banks) is the attnV accumulator.  4+2+2 = 8 banks.
        cp = ctx.enter_context(tc.tile_pool(name="const", bufs=1))
        psp = ctx.enter_context(tc.tile_pool(name="psum", bufs=2,
                                             space="PSUM"))
        pso = ctx.enter_context(tc.tile_pool(name="psum_o", bufs=1,
                                             space="PSUM"))
        p_pool = ctx.enter_context(tc.tile_pool(name="pp", bufs=3))
        u_pool = ctx.enter_context(tc.tile_pool(name="usb", bufs=2))
        nrm_b = ctx.enter_context(tc.tile_pool(name="nrm_b", bufs=2))
        nrm_d = ctx.enter_context(tc.tile_pool(name="nrm_d", bufs=2,
                                               space="DRAM"))
        out_pool = ctx.enter_context(tc.tile_pool(name="oute", bufs=2))

        xT_sb = cp.tile([P, KT, T], dtx)
        wq_sb = cp.tile([P, KT, C], dtx)
        wk_sb = cp.tile([P, KT, C], dtx)
        wv_sb = cp.tile([P, KT, C], dtx)
        we_sb = cp.tile([P, MT, D], dto)
        qT_sb = cp.tile([P, MT, T], dtqk)
        # kT is stored per-head zero-padded to the full 128 partitions
        # (head h's 64 rows sit at their natural partition offset, the
        # other 64 rows are zero).  Scores matmuls then run with K=128 so
        # the PE array registers full activity — narrow K=64 matmuls keep
        # the HAM clock gate throttled at 1.2 GHz for the whole attention
        # phase (measured), doubling every matmul.  Same trick for v_aug:
        # M padded 65 -> 128 with zero columns.
        kT_sb = cp.tile([P, HC, T], dtqk)
        va_sb = cp.tile([P, TT, HC, P], dtp)
        oT_sb = cp.tile([P, MT, T], dto)
        nc.vector.memset(kT_sb, 0.0)
        nc.vector.memset(va_sb, 0.0)

        # ---- loads (k/q weights first — they gate the first scores)
        nc.sync.dma_start(out=wk_sb, in_=wk.rearrange("(k p) c -> p k c", p=P))
        nc.sync.dma_start(out=wq_sb, in_=wq.rearrange("(k p) c -> p k c", p=P))
        for half in range(2):
            kk = KT // 2
            nc.sync.dma_start(
                out=xT_sb[:, half * kk:(half + 1) * kk, :],
                in_=xT[half * kk * P:(half + 1) * kk * P, :].rearrange(
                    "(k p) t -> p k t", p=P))
        nc.gpsimd.dma_start(out=wv_sb,
                            in_=wv.rearrange("(k p) c -> p k c", p=P))
        nc.gpsimd.dma_start(out=we_sb,
                            in_=we.rearrange("(m p) d -> p m d", p=P))

        def proj_qk(w_sb, m, jhs=(0, 1)):
            for jh in jhs:
                ps = psp.tile([P, T // 2], f32, tag="s")
                for k in range(KT):
                    for c0 in range(0, T // 2, nch(dtx)):
                        nc.tensor.matmul(
                            ps[:, c0:c0 + nch(dtx)],
                            lhsT=w_sb[:, k, m * P:(m + 1) * P],
                            rhs=xT_sb[:, k, jh * (T // 2) + c0:
                                      jh * (T // 2) + c0 + nch(dtx)],
                            start=(k == 0), stop=(k == KT - 1),
                        )
                sl = slice(jh * (T // 2), (jh + 1) * (T // 2))
                if w_sb is wq_sb:
                    nc.vector.tensor_copy(qT_sb[:, m, sl], ps)
                else:
                    # zero-padded per-head layout: each head's rows stay
                    # at their natural partition offset
                    nc.vector.tensor_copy(kT_sb[0:DH, 2 * m, sl], ps[0:DH, :])
                    nc.vector.tensor_copy(kT_sb[DH:P, 2 * m + 1, sl],
                                          ps[DH:P, :])

        def proj_v_tile(t):
            ps = psp.tile([P, C], f32, tag="v")
            for k in range(KT):
                nc.tensor.matmul(
                    ps,
                    lhsT=xT_sb[:, k, t * P:(t + 1) * P],
                    rhs=wv_sb[:, k, :],
                    start=(k == 0), stop=(k == KT - 1),
                )
            nc.vector.memset(va_sb[:, t, :, DH:DH + 1], 1.0)
            nc.vector.tensor_copy(
                va_sb[:, t, :, 0:DH],
                ps.rearrange("p (h d) -> p h d", h=HC))
            # columns DH+1..P stay zero (padding to M=128)

        def attention(jh, h, pre_kt=None):
            pb = (h % 2) * DH
            m = h // 2
            po = pso.tile([P, JW], f32, tag="o")
            for kt in range(TT):
                if pre_kt is not None:
                    pre_kt(kt)
                ps = psp.tile([P, JW], f32, tag="s")
                for c0 in range(0, JW, nch(dtqk)):
                    nc.tensor.matmul(
                        ps[:, c0:c0 + nch(dtqk)],
                        lhsT=kT_sb[:, h, kt * P:(kt + 1) * P],
                        rhs=qT_sb[:, m, jh * JW + c0:jh * JW + c0 + nch(dtqk)],
                        start=True, stop=True,
                    )
                p_t = p_pool.tile([P, JW], dtp, tag="p")
                nc.scalar.activation(p_t, ps, Exp)
                for c0 in range(0, JW, nch(dtp)):
                    nc.tensor.matmul(
                        po[:, c0:c0 + nch(dtp)],
                        lhsT=va_sb[:, kt, h, :],
                        rhs=p_t[:, c0:c0 + nch(dtp)],
                        start=(kt == 0), stop=(kt == TT - 1),
                    )
            # normalize. Stash [out; denom] in SBUF so the PSUM accumulator
            # frees immediately; the reciprocal + partition broadcast run
            # off the critical path.  DVE is per-lane so the denominator row
            # crosses partitions via a DRAM bounce; the (custom-DVE)
            # reciprocal must run at partition base 0 (HW quirk).
            u_sb = u_pool.tile([DH + 1, JW], f32, tag="u")
            nc.vector.tensor_copy(u_sb, po[0:DH + 1, :])
            r_dr = nrm_d.tile([1, JW], f32, tag="rd")
            nc.sync.dma_start(out=r_dr, in_=u_sb[DH:DH + 1, :])
            lbc = nrm_b.tile([DH, JW], f32, tag="lbc")
            nc.sync.dma_start(out=lbc, in_=r_dr.to_broadcast((DH, JW)))
            rbc = nrm_b.tile([DH, JW], f32, tag="rbc")
            nc.vector.reciprocal_approx_fast(rbc, lbc)
            if pb == 0:
                nc.vector.tensor_mul(
                    oT_sb[0:DH, m, jh * JW:(jh + 1) * JW], u_sb[0:DH, :], rbc)
            else:
                st = nrm_b.tile([DH, JW], dto, tag="st")
                nc.vector.tensor_mul(st, u_sb[0:DH, :], rbc)
                nc.sync.dma_start(
                    out=oT_sb[pb:pb + DH, m, jh * JW:(jh + 1) * JW], in_=st)

        def emerge(jh):
            # out[t, :] = sum_m oT[:, m, t-tile]^T @ we[m] for this query
            # half (all 4 heads of this jh must be in oT).
            for t in range(jh * TT // 2, (jh + 1) * TT // 2):
                pe = psp.tile([P, D], f32, tag="s")
                for m in range(MT):
                    for c0 in range(0, D, nch(dto)):
                        nc.tensor.matmul(
                            pe[:, c0:c0 + nch(dto)],
                            lhsT=oT_sb[:, m, t * P:(t + 1) * P],
                            rhs=we_sb[:, m, c0:c0 + nch(dto)],
                            start=(m == 0), stop=(m == MT - 1),
                        )
                ob = out_pool.tile([P, D], f32, tag="ob")
                if jh == 1 and t % 2 == 1:
                    nc.scalar.copy(ob, pe)   # tail half: ACT is idle
                else:
                    nc.vector.tensor_copy(ob, pe)
                eng = nc.sync if t % 2 == 0 else nc.gpsimd
                eng.dma_start(out=out[t * P:(t + 1) * P, :], in_=ob)

        # ---- program order = scheduler priority.  Attention for the m=0
        # heads is emitted right after the m=0 projections so exp starts
        # ~30us in; v/m=1 projections fill PE slack under the ACT-bound
        # attention.  jh=1 ends on an even head (short final normalize).
        proj_qk(wk_sb, 0)
        proj_qk(wq_sb, 0, jhs=(0,))
        # v-projection is interleaved tile-by-tile into head 0's kt loop:
        # va[kt] is written right before attnV(kt) reads it, so the first
        # exp fires as soon as the m=0 projections land.
        attention(0, 0, pre_kt=proj_v_tile)
        proj_qk(wq_sb, 0, jhs=(1,))
        attention(0, 1)
        proj_qk(wk_sb, 1)
        proj_qk(wq_sb, 1)
        attention(0, 2)
        attention(0, 3)
        emerge(0)
        for h in (1, 3, 2, 0):
            attention(1, h)
        emerge(1)

    nc.compile()
    return nc


def prep_inputs(x, Wq, Wk, Wv, We, beta, input_valve, output_valve, chi,
                cfg=CFG):
    """Host-side prep: fold chamber into We, fold 1/scale into Wq, shard."""
    x = np.asarray(x, np.float32)
    Wq = np.asarray(Wq, np.float32)
    Wk = np.asarray(Wk, np.float32)
    Wv = np.asarray(Wv, np.float32)
    We = np.asarray(We, np.float32)

    def sig(v):
        return 1.0 / (1.0 + np.exp(-np.asarray(v, np.float64)))

    b = sig(beta)
    iv = sig(input_valve)
    ov = sig(output_valve)
    g = np.tanh(np.asarray(chi, np.float64))
    ang = math.pi * b
    ca, sa = np.cos(ang), np.sin(ang)
    half = DH // 2

    We64 = We.astype(np.float64)
    WeP = np.empty((D, D), np.float64)
    for h in range(H):
        L = np.zeros((DH, DH))
        idx = np.arange(half)
        L[idx, idx] = ca[h]
        L[idx, half + idx] = -sa[h]
        L[half + idx, idx] = sa[h]
        L[half + idx, half + idx] = ca[h]
        L *= ov[h] * g[h] * iv[h]
        WeP[:, h * DH:(h + 1) * DH] = We64[:, h * DH:(h + 1) * DH] @ L

    dt_x = _np_dt(cfg["dt_x"])
    dt_o = _np_dt(cfg["dt_o"])
    WqT = np.ascontiguousarray((Wq.astype(np.float64) / SCALE).T, dt_x)
    WkT = np.ascontiguousarray(Wk.T, dt_x)
    WvT = np.ascontiguousarray(Wv.T, dt_x)
    WeT = np.ascontiguousarray(WeP.T, dt_o)   # [c, dout]

    in_maps = []
    for core in range(NCORES):
        bidx, grp = divmod(core, H // HC)
        cols = slice(grp * C, (grp + 1) * C)
        in_maps.append({
            "xt": np.ascontiguousarray(x[bidx].T.astype(dt_x)),
            "wq": np.ascontiguousarray(WqT[:, cols]),
            "wk": np.ascontiguousarray(WkT[:, cols]),
            "wv": np.ascontiguousarray(WvT[:, cols]),
            "we": np.ascontiguousarray(WeT[cols, :]),
        })
    return in_maps


def kernel(**inputs):
    global LAST_EXEC_NS
    import os
    if "nc" not in _CACHE:
        _CACHE["nc"] = build_nc()
    nc = _CACHE["nc"]
    in_maps = prep_inputs(**inputs)

    from concourse.bass_utils import run_bass_kernel_spmd
    trace = bool(os.environ.get("CIRC_TRACE"))
    res = run_bass_kernel_spmd(nc, in_maps, list(range(NCORES)), trace=trace)
    LAST_EXEC_NS = res.exec_time_ns
    _CACHE["last_results"] = res

    B = 2
    outp = np.zeros((B, T, D), np.float32)
    per_batch = NCORES // B
    for core in range(NCORES):
        outp[core // per_batch] += res.results[core]["out"]
    return outp
